# revision 18
# baseline (speedup 1.0000x reference)
"""Trainium2 Bass kernel for a DANet-style DualAttentionBlock.

Full-input contract: kernel(**inputs) takes the complete tensors and returns
the complete [4, 256, 64, 64] output.

Sharding: 8 NeuronCores = 4 samples x 2 row-halves (32 rows each). Each core:
  - computes q for its 34-row query window (32 real + 1 halo row each side;
    out-of-range halo rows are zero-padded on host and zeroed on device after
    the attention combine),
  - computes k/v for all 4096 key positions of its sample,
  - position attention in transposed orientation (keys on partitions) so the
    softmax denominator folds into matmuls and no attention transpose is
    needed,
  - full channel attention (softmax row-wise, then a 256x256 PE transpose;
    the symmetric energy matrix lets the lower-left block be a transpose of
    the upper-right),
  - the 3x3 fusion conv for its 32 output rows (halo rows give exact edges),
  - partial BN stats; two pipelined AllReduces across the 8 cores yield exact
    train-mode batch stats; normalize + ReLU on device.

Precision strategy: fp16 operands everywhere on the attention path (q/k/v/x
and channel energies; fp32 PSUM accumulation throughout), which keeps every
PE weight load at the fast 2-byte path; attention weights and the fusion conv
in bf16. The PV matmul runs in [query, channel] orientation against V
augmented with a ones column, so the softmax denominator falls out of column
256 for free; results are PE-transposed back into the conv layout. All small
per-partition vectors (biases, BN affine, halo masks) ride in one packed
[128, 12] tensor, and the matmul weights in one packed fp16 tensor, so the
startup-critical path is six large DMAs split across both hardware queues.
"""

import os
import sys
import types

for _p in ("/opt/trn_rl_repo",):
    if _p not in sys.path:
        sys.path.append(_p)

import numpy as np
import ml_dtypes  # noqa: F401

import concourse.bass as bass
import concourse.mybir as mybir
import concourse.tile as tile
from concourse import bacc
from concourse.bass_utils import run_bass_kernel_spmd

F32 = mybir.dt.float32
BF16 = mybir.dt.bfloat16
F16 = mybir.dt.float16
AF = mybir.ActivationFunctionType
ALU = mybir.AluOpType
AX = mybir.AxisListType

B, C, H, W = 4, 256, 64, 64
N = H * W              # 4096 key positions
WROWS = 34             # query-window rows (1 halo + 32 real + 1 halo)
WQ = WROWS * W         # 2176 window positions
BN_EPS = 1e-5
NPOS = float(B * H * W)  # BN normalizer (16384)

# i-chunks for the attention phase
CHUNKS = [(0, 512), (512, 512), (1024, 512), (1536, 384), (1920, 256)]

# packed weight tensor column offsets (all fp16); ct=1 block lives at +1152
W_Q, W_K, W_V, IDN = 0, 128, 256, 512
CT1 = 1152
WPACK_COLS = 2304

# packed smalls column offsets (f32)
S_BQ, S_BK = 0, 1
S_GBV0, S_GBV1 = 2, 3
S_BNG0, S_BNG1 = 4, 5
S_BNB0, S_BNB1 = 6, 7
S_MTOP, S_MBOT = 8, 9
S_EPS = 10
SMALL_COLS = 11

LAST_RESULT = {}


def _build(gpa: float, gca: float):
    nc = bacc.Bacc("TRN2", target_bir_lowering=False)

    xs_d = nc.dram_tensor("xs", [C, N], F16, kind="ExternalInput")
    xq_d = nc.dram_tensor("xq", [C, WQ], F16, kind="ExternalInput")
    wpack_d = nc.dram_tensor("wpack", [128, WPACK_COLS], F16, kind="ExternalInput")
    smalls_d = nc.dram_tensor("smalls", [128, SMALL_COLS], F32, kind="ExternalInput")
    wfp_d = nc.dram_tensor("wfp", [128, 4 * 2304], BF16, kind="ExternalInput")
    out_d = nc.dram_tensor("out", [C, 2048], F32, kind="ExternalOutput")

    stats_in_d = [nc.dram_tensor(f"stats_in{o}", [128, 2], F32) for o in range(2)]
    stats_out_d = [nc.dram_tensor(f"stats_out{o}", [128, 2], F32,
                                  addr_space="Shared") for o in range(2)]

    with tile.TileContext(nc) as tc:
        with (
            tc.tile_pool(name="consts", bufs=1) as consts,
            tc.tile_pool(name="work", bufs=1) as work,
            tc.tile_pool(name="persist", bufs=1) as persist,
        ):
            # long-lived activation tensors for the attention phase
            with tc.tile_pool(name="bigC", bufs=1) as bigC:
                qrep = bigC.tile([128, WQ], F16, tag="qrep", name="qrep")
                krep = bigC.tile([128, N], F16, tag="krep", name="krep")
                vT = bigC.tile([128, 32, 257], BF16, tag="vT", name="vT")
                A = persist.tile([128, 2, C], F16, tag="A", name="A")
                grca = [None, None]
                xqr = [None, None]

                # ---------- phase A/B: input DMA, qkv gen, xT, channel attn ----
                with (
                    tc.tile_pool(name="bigA", bufs=1) as bigA,
                    tc.tile_pool(name="psAB", bufs=1, space="PSUM") as psAB,
                ):
                    # startup-critical loads, split over both HWDGE queues;
                    # x halves are split along n so k/v-gen can start on the
                    # first half while the second still streams
                    smalls = consts.tile([128, SMALL_COLS], F32, tag="smalls",
                                         name="smalls")
                    nc.sync.dma_start(out=smalls[:], in_=smalls_d[:])
                    wpk = consts.tile([128, WPACK_COLS], F16, tag="wpk", name="wpk")
                    nc.sync.dma_start(out=wpk[:], in_=wpack_d[:])
                    for ct in range(2):
                        xqr[ct] = bigC.tile([128, WQ], F16, tag=f"xqr{ct}",
                                            name=f"xqr{ct}")
                    nc.scalar.dma_start(out=xqr[0][:], in_=xq_d[0:128, :])
                    nc.scalar.dma_start(out=xqr[1][:], in_=xq_d[128:256, :])
                    xr = [None, None]
                    for ct in range(2):
                        xr[ct] = bigA.tile([128, N], F16, tag=f"xr{ct}",
                                           name=f"xr{ct}")
                    nc.sync.dma_start(out=xr[0][:, 0:2048], in_=xs_d[0:128, 0:2048])
                    nc.scalar.dma_start(out=xr[1][:, 0:2048],
                                        in_=xs_d[128:256, 0:2048])
                    nc.sync.dma_start(out=xr[0][:, 2048:4096],
                                      in_=xs_d[0:128, 2048:4096])
                    nc.scalar.dma_start(out=xr[1][:, 2048:4096],
                                        in_=xs_d[128:256, 2048:4096])

                    wq_r = [wpk[:, ct * CT1 + W_Q : ct * CT1 + W_Q + 128]
                            for ct in range(2)]
                    wk_r = [wpk[:, ct * CT1 + W_K : ct * CT1 + W_K + 128]
                            for ct in range(2)]
                    wv_r = [wpk[:, ct * CT1 + W_V : ct * CT1 + W_V + C]
                            for ct in range(2)]
                    ident = wpk[:, IDN:IDN + 128]

                    # conv weights prefetch (needed only in phase D)
                    wfp_sb = persist.tile([128, 4 * 2304], BF16, tag="wfp",
                                          name="wfp")
                    nc.scalar.dma_start(out=wfp_sb[:], in_=wfp_d[:])

                    # ones column of augmented V: softmax denominator source
                    nc.vector.memset(vT[:, :, 256:257], 1.0)

                    # q/k generation (4x-replicated along d)
                    for off, cw in CHUNKS:
                        ps = psAB.tile([128, cw], F32, tag="qk", name="qk", bufs=2)
                        nc.tensor.matmul(ps[:], wq_r[0], xqr[0][:, off : off + cw],
                                         start=True, stop=False)
                        nc.tensor.matmul(ps[:], wq_r[1], xqr[1][:, off : off + cw],
                                         start=False, stop=True)
                        nc.scalar.activation(qrep[:, off : off + cw], ps[:],
                                             AF.Identity,
                                             bias=smalls[:, S_BQ:S_BQ + 1],
                                             scale=0.25)
                    for kc in range(8):
                        off = 512 * kc
                        ps = psAB.tile([128, 512], F32, tag="qk", name="qk", bufs=2)
                        nc.tensor.matmul(ps[:], wk_r[0], xr[0][:, off : off + 512],
                                         start=True, stop=False)
                        nc.tensor.matmul(ps[:], wk_r[1], xr[1][:, off : off + 512],
                                         start=False, stop=True)
                        nc.scalar.activation(krep[:, off : off + 512], ps[:],
                                             AF.Identity,
                                             bias=smalls[:, S_BK:S_BK + 1],
                                             scale=1.0)

                    # vT gen + x transpose -> channel-attention energy
                    # ec[1] accumulates only its diagonal block; the off-diag
                    # block arrives by transposing ec[0]'s right half.
                    ec = [psAB.tile([128, C], F32, tag=f"ec{ih}", name=f"ec{ih}")
                          for ih in range(2)]
                    for nt in range(32):
                        sl = slice(128 * nt, 128 * nt + 128)
                        ps = psAB.tile([128, C], F32, tag="vx", name="vx", bufs=2)
                        nc.tensor.matmul(ps[:], xr[0][:, sl], wv_r[0],
                                         start=True, stop=False)
                        nc.tensor.matmul(ps[:], xr[1][:, sl], wv_r[1],
                                         start=False, stop=True)
                        nc.vector.tensor_copy(vT[:, nt, 0:256], ps[:])
                        ps2 = psAB.tile([128, C], F16, tag="xt", name="xt", bufs=2)
                        nc.tensor.transpose(ps2[:, 0:128], xr[0][:, sl], ident)
                        nc.tensor.transpose(ps2[:, 128:256], xr[1][:, sl], ident)
                        xTn = bigA.tile([128, C], F16, tag="xTn", name="xTn",
                                        bufs=2)
                        nc.vector.tensor_copy(xTn[:], ps2[:])
                        nc.tensor.matmul(ec[0][:], xTn[:, 0:128], xTn[:],
                                         start=(nt == 0), stop=(nt == 31))
                        nc.tensor.matmul(ec[1][:, 128:256], xTn[:, 128:256],
                                         xTn[:, 128:256],
                                         start=(nt == 0), stop=(nt == 31))

                    # fill ec[1]'s off-diagonal block by symmetry
                    ecsb = work.tile([128, 128], F32, tag="ecsb", name="ecsb")
                    nc.vector.tensor_copy(ecsb[:], ec[0][:, 128:256])
                    identf = work.tile([128, 128], F32, tag="identf", name="identf")
                    nc.vector.tensor_copy(identf[:], ident)
                    nc.tensor.transpose(ec[1][:, 0:128], ecsb[:], identf[:])

                    # channel-attention softmax (row-wise) + transpose
                    U = [None, None]
                    for ih in range(2):
                        negmax = work.tile([128, 1], F32, tag="negmax",
                                           name="negmax")
                        nc.vector.reduce_max(negmax[:], ec[ih][:], axis=AX.X,
                                             negate=True)
                        U[ih] = bigA.tile([128, C], F16, tag=f"U{ih}",
                                          name=f"U{ih}")
                        nc.scalar.activation(U[ih][:], ec[ih][:], AF.Exp,
                                             bias=negmax[:], scale=1.0)
                        ssum = work.tile([128, 1], F32, tag="ssum", name="ssum")
                        nc.vector.reduce_sum(ssum[:], U[ih][:], axis=AX.X)
                        rc = work.tile([128, 1], F32, tag="rc", name="rc")
                        nc.vector.reciprocal(rc[:], ssum[:])
                        grca[ih] = consts.tile([128, 1], F32, tag=f"grca{ih}",
                                               name=f"grca{ih}")
                        nc.vector.tensor_scalar_mul(grca[ih][:], rc[:], gca)
                    for ih in range(2):
                        for jt in range(2):
                            trp = psAB.tile([128, 128], F16, tag="xt", name="utr",
                                            bufs=2)
                            nc.tensor.transpose(
                                trp[:], U[ih][:, 128 * jt : 128 * jt + 128],
                                ident)
                            nc.vector.tensor_copy(A[:, jt, 128 * ih : 128 * ih + 128],
                                                  trp[:])

                # ---------- phase C: channel-attn apply + position attention ----
                pad = [persist.tile([128, WROWS, W + 2], BF16, tag=f"pad{t}",
                                    name=f"pad{t}")
                       for t in range(4)]
                # only the two border columns need zeroing: the finalize ops
                # write every interior column of all 34 rows
                zcol = work.tile([128, WROWS, 1], F32, tag="zcol", name="zcol")
                nc.vector.memset(zcol[:], 0.0)
                for t in range(4):
                    nc.vector.tensor_copy(pad[t][:, :, 0:1], zcol[:])
                    nc.vector.tensor_copy(pad[t][:, :, W + 1 : W + 2], zcol[:])

                with (
                    tc.tile_pool(name="psC", bufs=1, space="PSUM") as psC,
                    tc.tile_pool(name="ptp", bufs=2) as ptp,
                ):
                    # ca = (U @ xq) * (gamma_ca / rowsum) + xq, into padded tiles
                    for ih in range(2 * (not os.environ.get("KERNEL_SKIP_CA"))):
                        isl = slice(128 * ih, 128 * ih + 128)
                        for off, cw in CHUNKS:
                            rows = cw // W
                            roff = off // W
                            ca = psC.tile([128, cw], F32, tag=f"pa{ih}",
                                          name=f"pa{ih}")
                            nc.tensor.matmul(ca[:], A[:, 0, isl],
                                             xqr[0][:, off : off + cw],
                                             start=True, stop=False)
                            nc.tensor.matmul(ca[:], A[:, 1, isl],
                                             xqr[1][:, off : off + cw],
                                             start=False, stop=True)
                            nc.vector.scalar_tensor_tensor(
                                out=pad[2 + ih][:, roff : roff + rows, 1 : 1 + W],
                                in0=ca[:].rearrange("p (r w) -> p r w", w=W),
                                scalar=grca[ih][:],
                                in1=xqr[ih][:, off : off + cw]
                                    .rearrange("p (r w) -> p r w", w=W),
                                op0=ALU.mult, op1=ALU.add,
                            )

                    for off, cw in ([] if os.environ.get("KERNEL_SKIP_C") else CHUNKS):
                        PT = ptp.tile([128, 32, 512], BF16, tag="pt", name="pt")
                        for g in range(16):
                            eg = psC.tile([128, 2, 512], F32, tag=f"eg{g % 2}",
                                          name=f"eg{g % 2}")
                            for jj in range(2):
                                jt = 2 * g + jj
                                for c0 in range(0, cw, 256):
                                    ce = min(c0 + 256, cw)
                                    nc.tensor.matmul(
                                        eg[:, jj, c0:ce],
                                        krep[:, 128 * jt : 128 * jt + 128],
                                        qrep[:, off + c0 : off + ce],
                                        start=True, stop=True,
                                    )
                            nc.scalar.activation(PT[:, 2 * g : 2 * g + 2, 0:cw],
                                                 eg[:, :, 0:cw], AF.Exp,
                                                 bias=0.0, scale=1.0)

                        for ib in range(cw // 128):
                            gib = off // 128 + ib
                            paps = psC.tile([128, 257], F32, tag=f"pa{ib % 3}",
                                            name=f"pa{ib % 3}")
                            for jt in range(32):
                                nc.tensor.matmul(
                                    paps[:],
                                    PT[:, jt, 128 * ib : 128 * ib + 128],
                                    vT[:, jt, :],
                                    start=(jt == 0), stop=(jt == 31),
                                )
                            recip = work.tile([128, 1], F32, tag="recip",
                                              name="recip", bufs=2)
                            nc.vector.reciprocal(recip[:], paps[:, 256:257])
                            grm = work.tile([128, 1], F32, tag="grm", name="grm",
                                            bufs=2)
                            nc.vector.tensor_scalar_mul(grm[:], recip[:], gpa)
                            tsc = work.tile([128, C], F16, tag="tsc", name="tsc",
                                            bufs=2)
                            nc.vector.tensor_scalar_mul(tsc[:], paps[:, 0:C],
                                                        grm[:])
                            for ch in range(2):
                                trp = psC.tile([128, 128], F16, tag="tr",
                                               name="tr")
                                nc.tensor.transpose(
                                    trp[:], tsc[:, 128 * ch : 128 * ch + 128],
                                    ident)
                                r2 = 128 // W
                                r0 = gib * r2
                                gbv = smalls[:, S_GBV0 + ch : S_GBV0 + ch + 1]
                                nc.vector.scalar_tensor_tensor(
                                    out=pad[ch][:, r0 : r0 + r2, 1 : 1 + W],
                                    in0=trp[:].rearrange("p (r w) -> p r w", w=W),
                                    scalar=gbv,
                                    in1=xqr[ch][:, 128 * gib : 128 * gib + 128]
                                        .rearrange("p (r w) -> p r w", w=W),
                                    op0=ALU.add, op1=ALU.add,
                                )

                    # zero out-of-image halo rows of the position-attention pads
                    # (they carry bias terms from the zero-padded xq window)
                    for ch in range(2):
                        nc.vector.tensor_scalar_mul(
                            pad[ch][:, 0:1, :], pad[ch][:, 0:1, :],
                            smalls[:, S_MTOP:S_MTOP + 1])
                        nc.vector.tensor_scalar_mul(
                            pad[ch][:, WROWS - 1 : WROWS, :],
                            pad[ch][:, WROWS - 1 : WROWS, :],
                            smalls[:, S_MBOT:S_MBOT + 1])

            # ---------- phase D: 3x3 conv + BN stats ----------
            y_sb = [persist.tile([128, 2048], F32, tag=f"ysb{o}", name=f"ysb{o}")
                    for o in range(2)]
            allst = [None, None]
            if os.environ.get("KERNEL_SKIP_D"):
                for o in range(2):
                    nc.vector.memset(y_sb[o][:], 0.0)
            sums = [consts.tile([128, 4], F32, tag=f"sums{o}", name=f"sums{o}")
                    for o in range(2)]
            sqs = [consts.tile([128, 4], F32, tag=f"sqs{o}", name=f"sqs{o}")
                   for o in range(2)]
            if os.environ.get("KERNEL_SKIP_D"):
                for o in range(2):
                    nc.vector.memset(sums[o][:], 0.0)
                    nc.vector.memset(sqs[o][:], 0.0)

            with (
                tc.tile_pool(name="psD", bufs=4, space="PSUM") as psD,
            ):
                for oh in range(2 * (not os.environ.get("KERNEL_SKIP_D"))):
                    for pc in range(4):
                        # the two 4-row halves accumulate in separate PSUM
                        # banks: interleaved open accumulation groups must not
                        # share a bank
                        yps = psD.tile([128, 2, 512], F32, tag="y", name="y",
                                       bufs=2)
                        first = True
                        for it in range(4):
                            for dy in range(3):
                                for dx in range(3):
                                    woff = (it * 18 + (dy * 3 + dx) * 2 + oh) * 128
                                    last = (it == 3 and dy == 2 and dx == 2)
                                    for hb in range(2):
                                        r0 = 8 * pc + dy + 4 * hb
                                        nc.tensor.matmul(
                                            yps[:, hb, 0:256],
                                            wfp_sb[:, woff : woff + 128],
                                            pad[it][:, r0 : r0 + 4, dx : dx + W],
                                            start=first, stop=last,
                                        )
                                    first = False
                        yv = yps[:, :, 0:256]
                        ysl = (y_sb[oh][:, 512 * pc : 512 * pc + 512]
                               .rearrange("p (h c) -> p h c", h=2))
                        nc.scalar.copy(ysl, yv)
                        nc.vector.reduce_sum(sums[oh][:, pc : pc + 1], yv,
                                             axis=AX.XY)
                        dscr = work.tile([128, 2, 256], F32, tag="dscr",
                                         name="dscr", bufs=2)
                        nc.scalar.activation(dscr[:], yv, AF.Square,
                                             accum_out=sqs[oh][:, pc : pc + 1])

                    stats_sb = consts.tile([128, 2], F32, tag=f"stats{oh}",
                                           name=f"stats{oh}")
                    nc.vector.reduce_sum(stats_sb[:, 0:1], sums[oh][:], axis=AX.X)
                    nc.vector.reduce_sum(stats_sb[:, 1:2], sqs[oh][:], axis=AX.X)
                    nc.sync.dma_start(out=stats_in_d[oh][:], in_=stats_sb[:])
                    # oh=0's AllReduce overlaps oh=1's conv half
                    nc.gpsimd.collective_compute(
                        "AllReduce", ALU.add,
                        replica_groups=[list(range(8))],
                        ins=[stats_in_d[oh][:]],
                        outs=[stats_out_d[oh][:]],
                    )
                    allst[oh] = consts.tile([128, 2], F32, tag=f"allst{oh}",
                                            name=f"allst{oh}")
                    nc.sync.dma_start(out=allst[oh][:], in_=stats_out_d[oh][:])

            scale_t = [None, None]
            shift_t = [None, None]
            for oh in range(2):
                mean = work.tile([128, 1], F32, tag="mean", name="mean")
                nc.vector.tensor_scalar_mul(mean[:], allst[oh][:, 0:1], 1.0 / NPOS)
                ex2 = work.tile([128, 1], F32, tag="ex2", name="ex2")
                nc.vector.tensor_scalar_mul(ex2[:], allst[oh][:, 1:2], 1.0 / NPOS)
                msq = work.tile([128, 1], F32, tag="msq", name="msq")
                nc.vector.tensor_mul(msq[:], mean[:], mean[:])
                var = work.tile([128, 1], F32, tag="var", name="var")
                nc.vector.tensor_sub(var[:], ex2[:], msq[:])
                std = work.tile([128, 1], F32, tag="std", name="std")
                nc.scalar.activation(std[:], var[:], AF.Sqrt,
                                     bias=smalls[:, S_EPS:S_EPS + 1], scale=1.0)
                rstd = work.tile([128, 1], F32, tag="rstd", name="rstd")
                nc.vector.reciprocal(rstd[:], std[:])
                scale_t[oh] = consts.tile([128, 1], F32, tag=f"scale{oh}",
                                          name=f"scale{oh}")
                nc.vector.tensor_mul(scale_t[oh][:],
                                     smalls[:, S_BNG0 + oh : S_BNG0 + oh + 1],
                                     rstd[:])
                tmp = work.tile([128, 1], F32, tag="tmp", name="tmp")
                nc.vector.tensor_mul(tmp[:], mean[:], scale_t[oh][:])
                shift_t[oh] = consts.tile([128, 1], F32, tag=f"shift{oh}",
                                          name=f"shift{oh}")
                nc.vector.tensor_sub(shift_t[oh][:],
                                     smalls[:, S_BNB0 + oh : S_BNB0 + oh + 1],
                                     tmp[:])

            for oh in range(2):
                for pc in range(4):
                    rsb = work.tile([128, 512], F32, tag="rsb", name="rsb", bufs=2)
                    nc.vector.tensor_scalar(
                        out=rsb[:], in0=y_sb[oh][:, 512 * pc : 512 * pc + 512],
                        scalar1=scale_t[oh][:], scalar2=shift_t[oh][:],
                        op0=ALU.mult, op1=ALU.add)
                    osb = work.tile([128, 512], F32, tag="osb", name="osb", bufs=2)
                    nc.vector.tensor_scalar_max(osb[:], rsb[:], 0.0)
                    nc.sync.dma_start(
                        out=out_d[128 * oh : 128 * oh + 128,
                                  512 * pc : 512 * pc + 512],
                        in_=osb[:],
                    )

    nc.compile()
    return nc


def _ensure_trace_hook():
    try:
        import antenv.axon_hooks  # noqa: F401
        return
    except ImportError:
        pass
    try:
        from trn_agent_boot.trn_boot import _ntff_profile_via_ctypes
    except ImportError:
        return
    mod = types.ModuleType("antenv.axon_hooks")
    try:
        hook = _ntff_profile_via_ctypes("/opt/axon/libaxon_pjrt.so")
    except Exception:
        return
    mod.get_axon_ntff_profile_hook = lambda: hook
    mod.set_axon_ntff_profile_hook = lambda h: None
    sys.modules["antenv.axon_hooks"] = mod


def kernel(x, wq, bq, wk, bk, wv, bv, gamma_pa, gamma_ca, wf, bn_gamma, bn_beta):
    x = np.ascontiguousarray(np.asarray(x, np.float32))
    wq = np.asarray(wq, np.float32)
    bq = np.asarray(bq, np.float32)
    wk = np.asarray(wk, np.float32)
    bk = np.asarray(bk, np.float32)
    wv = np.asarray(wv, np.float32)
    bv = np.asarray(bv, np.float32)
    wf = np.asarray(wf, np.float32)
    gpa = float(np.asarray(gamma_pa).reshape(-1)[0])
    gca = float(np.asarray(gamma_ca).reshape(-1)[0])
    bn_gamma = np.asarray(bn_gamma, np.float32)
    bn_beta = np.asarray(bn_beta, np.float32)

    nc = _build(gpa, gca)

    # shared (per-core-identical) packed weights, all fp16
    wqrep = np.tile(wq.T, (1, 4))                # [256, 128]
    wkrep = np.tile(wk.T, (1, 4))
    wvt = wv.T                                   # [256, 256]
    wpack = np.zeros((128, WPACK_COLS), np.float32)
    for ct in range(2):
        rows = slice(128 * ct, 128 * ct + 128)
        wpack[:, ct * CT1 + W_Q : ct * CT1 + W_Q + 128] = wqrep[rows]
        wpack[:, ct * CT1 + W_K : ct * CT1 + W_K + 128] = wkrep[rows]
        wpack[:, ct * CT1 + W_V : ct * CT1 + W_V + C] = wvt[rows]
    wpack[:, IDN:IDN + 128] = np.eye(128, dtype=np.float32)
    wpack_h = np.ascontiguousarray(wpack).astype(np.float16)

    # wfp[i, it*2304 + (dy*3+dx)*2*128 + oh*128 + o'] = wf[o, 128*it + i, dy, dx]
    wft = np.ascontiguousarray(
        wf.reshape(C, 4, 128, 3, 3).transpose(1, 2, 3, 4, 0).reshape(4, 128, 2304))
    wfp_h = np.ascontiguousarray(
        wft.transpose(1, 0, 2).reshape(128, 4 * 2304)).astype(ml_dtypes.bfloat16)

    xh = x.astype(np.float16)

    in_maps = []
    for core in range(8):
        b, hf = divmod(core, 2)
        r0 = hf * 32
        e0 = r0 - 1
        xq = np.zeros((C, WROWS, W), np.float16)
        lo, hi = max(e0, 0), min(e0 + WROWS, H)
        xq[:, lo - e0 : hi - e0, :] = xh[b][:, lo:hi, :]
        smalls = np.zeros((128, SMALL_COLS), np.float32)
        smalls[:, S_BQ] = np.tile(bq, 4) / 4.0
        smalls[:, S_BK] = np.tile(bk, 4)
        smalls[:, S_GBV0] = gpa * bv[0:128]
        smalls[:, S_GBV1] = gpa * bv[128:256]
        smalls[:, S_BNG0] = bn_gamma[0:128]
        smalls[:, S_BNG1] = bn_gamma[128:256]
        smalls[:, S_BNB0] = bn_beta[0:128]
        smalls[:, S_BNB1] = bn_beta[128:256]
        smalls[:, S_MTOP] = 0.0 if hf == 0 else 1.0
        smalls[:, S_MBOT] = 1.0 if hf == 0 else 0.0
        smalls[:, S_EPS] = BN_EPS
        in_maps.append({
            "xs": np.ascontiguousarray(xh[b].reshape(C, N)),
            "xq": np.ascontiguousarray(xq.reshape(C, WQ)),
            "wpack": wpack_h,
            "smalls": np.ascontiguousarray(smalls),
            "wfp": wfp_h,
        })

    trace = bool(os.environ.get("BASS_TRACE"))
    if trace:
        _ensure_trace_hook()
    res = run_bass_kernel_spmd(nc, in_maps, list(range(8)), trace=trace)
    LAST_RESULT["exec_time_ns"] = res.exec_time_ns
    LAST_RESULT["mean_exec_time_ns"] = res.mean_exec_time_ns

    out = np.empty((B, C, H, W), np.float32)
    for core in range(8):
        b, hf = divmod(core, 2)
        out[b][:, 32 * hf : 32 * hf + 32, :] = (
            res.results[core]["out"].reshape(C, 32, W)
        )
    return out


# revision 20
# speedup vs baseline: 1.0121x; 1.0121x over previous
"""Trainium2 Bass kernel for a DANet-style DualAttentionBlock.

Full-input contract: kernel(**inputs) takes the complete tensors and returns
the complete [4, 256, 64, 64] output.

Sharding: 8 NeuronCores = 4 samples x 2 row-halves (32 rows each). Each core:
  - computes q for its 34-row query window (32 real + 1 halo row each side;
    out-of-range halo rows are zero-padded on host and zeroed on device after
    the attention combine),
  - computes k/v for all 4096 key positions of its sample,
  - position attention in transposed orientation (keys on partitions) so the
    softmax denominator folds into matmuls and no attention transpose is
    needed,
  - full channel attention (softmax row-wise, then a 256x256 PE transpose;
    the symmetric energy matrix lets the lower-left block be a transpose of
    the upper-right),
  - the 3x3 fusion conv for its 32 output rows (halo rows give exact edges),
  - partial BN stats; two pipelined AllReduces across the 8 cores yield exact
    train-mode batch stats; normalize + ReLU on device.

Precision strategy: fp16 operands everywhere on the attention path (q/k/v/x
and channel energies; fp32 PSUM accumulation throughout), which keeps every
PE weight load at the fast 2-byte path; attention weights and the fusion conv
in bf16. The PV matmul runs in [query, channel] orientation against V
augmented with a ones column, so the softmax denominator falls out of column
256 for free; results are PE-transposed back into the conv layout. All small
per-partition vectors (biases, BN affine, halo masks) ride in one packed
[128, 12] tensor, and the matmul weights in one packed fp16 tensor, so the
startup-critical path is six large DMAs split across both hardware queues.
"""

import os
import sys
import types

for _p in ("/opt/trn_rl_repo",):
    if _p not in sys.path:
        sys.path.append(_p)

import numpy as np
import ml_dtypes  # noqa: F401

import concourse.bass as bass
import concourse.mybir as mybir
import concourse.tile as tile
from concourse import bacc
from concourse.bass_utils import run_bass_kernel_spmd

F32 = mybir.dt.float32
BF16 = mybir.dt.bfloat16
F16 = mybir.dt.float16
AF = mybir.ActivationFunctionType
ALU = mybir.AluOpType
AX = mybir.AxisListType

B, C, H, W = 4, 256, 64, 64
N = H * W              # 4096 key positions
WROWS = 34             # query-window rows (1 halo + 32 real + 1 halo)
WQ = WROWS * W         # 2176 window positions
BN_EPS = 1e-5
NPOS = float(B * H * W)  # BN normalizer (16384)

# i-chunks for the attention phase
CHUNKS = [(0, 512), (512, 512), (1024, 512), (1536, 384), (1920, 256)]

# packed weight tensor column offsets (all fp16); ct=1 block lives at +1152
W_Q, W_K, W_V, IDN = 0, 128, 256, 512
CT1 = 1152
WPACK_COLS = 2304

# packed smalls column offsets (f32)
S_BQ, S_BK = 0, 1
S_GBV0, S_GBV1 = 2, 3
S_BNG0, S_BNG1 = 4, 5
S_BNB0, S_BNB1 = 6, 7
S_MTOP, S_MBOT = 8, 9
S_EPS = 10
SMALL_COLS = 11

LAST_RESULT = {}


def _build(gpa: float, gca: float):
    nc = bacc.Bacc("TRN2", target_bir_lowering=False)

    xs_d = nc.dram_tensor("xs", [C, N], F16, kind="ExternalInput")
    xq_d = nc.dram_tensor("xq", [C, WQ], F16, kind="ExternalInput")
    wpack_d = nc.dram_tensor("wpack", [128, WPACK_COLS], F16, kind="ExternalInput")
    smalls_d = nc.dram_tensor("smalls", [128, SMALL_COLS], F32, kind="ExternalInput")
    wfp_d = nc.dram_tensor("wfp", [128, 4 * 2304], BF16, kind="ExternalInput")
    out_d = nc.dram_tensor("out", [C, 2048], F32, kind="ExternalOutput")

    stats_in_d = [nc.dram_tensor(f"stats_in{o}", [128, 2], F32) for o in range(2)]
    stats_out_d = [nc.dram_tensor(f"stats_out{o}", [128, 2], F32,
                                  addr_space="Shared") for o in range(2)]

    with tile.TileContext(nc) as tc:
        with (
            tc.tile_pool(name="consts", bufs=1) as consts,
            tc.tile_pool(name="work", bufs=1) as work,
            tc.tile_pool(name="persist", bufs=1) as persist,
        ):
            # long-lived activation tensors for the attention phase
            with tc.tile_pool(name="bigC", bufs=1) as bigC:
                qrep = bigC.tile([128, WQ], F16, tag="qrep", name="qrep")
                krep = bigC.tile([128, N], F16, tag="krep", name="krep")
                vT = bigC.tile([128, 32, 257], BF16, tag="vT", name="vT")
                A = persist.tile([128, 2, C], F16, tag="A", name="A")
                grca = [None, None]
                xqr = [None, None]

                # ---------- phase A/B: input DMA, qkv gen, xT, channel attn ----
                with (
                    tc.tile_pool(name="bigA", bufs=1) as bigA,
                    tc.tile_pool(name="psAB", bufs=1, space="PSUM") as psAB,
                ):
                    # startup-critical loads, split over both HWDGE queues;
                    # x halves are split along n so k/v-gen can start on the
                    # first half while the second still streams
                    smalls = consts.tile([128, SMALL_COLS], F32, tag="smalls",
                                         name="smalls")
                    nc.sync.dma_start(out=smalls[:], in_=smalls_d[:])
                    wpk = consts.tile([128, WPACK_COLS], F16, tag="wpk", name="wpk")
                    nc.sync.dma_start(out=wpk[:], in_=wpack_d[:])
                    for ct in range(2):
                        xqr[ct] = bigC.tile([128, WQ], F16, tag=f"xqr{ct}",
                                            name=f"xqr{ct}")
                    nc.scalar.dma_start(out=xqr[0][:], in_=xq_d[0:128, :])
                    nc.scalar.dma_start(out=xqr[1][:], in_=xq_d[128:256, :])
                    xr = [None, None]
                    for ct in range(2):
                        xr[ct] = bigA.tile([128, N], F16, tag=f"xr{ct}",
                                           name=f"xr{ct}")
                    nc.sync.dma_start(out=xr[0][:, 0:2048], in_=xs_d[0:128, 0:2048])
                    nc.scalar.dma_start(out=xr[1][:, 0:2048],
                                        in_=xs_d[128:256, 0:2048])
                    nc.sync.dma_start(out=xr[0][:, 2048:4096],
                                      in_=xs_d[0:128, 2048:4096])
                    nc.scalar.dma_start(out=xr[1][:, 2048:4096],
                                        in_=xs_d[128:256, 2048:4096])

                    wq_r = [wpk[:, ct * CT1 + W_Q : ct * CT1 + W_Q + 128]
                            for ct in range(2)]
                    wk_r = [wpk[:, ct * CT1 + W_K : ct * CT1 + W_K + 128]
                            for ct in range(2)]
                    wv_r = [wpk[:, ct * CT1 + W_V : ct * CT1 + W_V + C]
                            for ct in range(2)]
                    ident = wpk[:, IDN:IDN + 128]

                    # conv weights prefetch (needed only in phase D)
                    wfp_sb = persist.tile([128, 4 * 2304], BF16, tag="wfp",
                                          name="wfp")
                    nc.scalar.dma_start(out=wfp_sb[:], in_=wfp_d[:])

                    # ones column of augmented V: softmax denominator source
                    nc.vector.memset(vT[:, :, 256:257], 1.0)

                    # q/k generation (4x-replicated along d)
                    for off, cw in CHUNKS:
                        ps = psAB.tile([128, cw], F32, tag="qk", name="qk", bufs=2)
                        nc.tensor.matmul(ps[:], wq_r[0], xqr[0][:, off : off + cw],
                                         start=True, stop=False)
                        nc.tensor.matmul(ps[:], wq_r[1], xqr[1][:, off : off + cw],
                                         start=False, stop=True)
                        nc.scalar.activation(qrep[:, off : off + cw], ps[:],
                                             AF.Identity,
                                             bias=smalls[:, S_BQ:S_BQ + 1],
                                             scale=0.25)
                    for kc in range(8):
                        off = 512 * kc
                        ps = psAB.tile([128, 512], F32, tag="qk", name="qk", bufs=2)
                        nc.tensor.matmul(ps[:], wk_r[0], xr[0][:, off : off + 512],
                                         start=True, stop=False)
                        nc.tensor.matmul(ps[:], wk_r[1], xr[1][:, off : off + 512],
                                         start=False, stop=True)
                        nc.scalar.activation(krep[:, off : off + 512], ps[:],
                                             AF.Identity,
                                             bias=smalls[:, S_BK:S_BK + 1],
                                             scale=1.0)

                    # vT gen + x transpose -> channel-attention energy
                    # ec[1] accumulates only its diagonal block; the off-diag
                    # block arrives by transposing ec[0]'s right half.
                    ec = [psAB.tile([128, C], F32, tag=f"ec{ih}", name=f"ec{ih}")
                          for ih in range(2)]
                    for nt in range(32):
                        sl = slice(128 * nt, 128 * nt + 128)
                        ps = psAB.tile([128, C], F32, tag="vx", name="vx", bufs=2)
                        nc.tensor.matmul(ps[:], xr[0][:, sl], wv_r[0],
                                         start=True, stop=False)
                        nc.tensor.matmul(ps[:], xr[1][:, sl], wv_r[1],
                                         start=False, stop=True)
                        nc.vector.tensor_copy(vT[:, nt, 0:256], ps[:])
                        ps2 = psAB.tile([128, C], F16, tag="xt", name="xt", bufs=2)
                        nc.tensor.transpose(ps2[:, 0:128], xr[0][:, sl], ident)
                        nc.tensor.transpose(ps2[:, 128:256], xr[1][:, sl], ident)
                        xTn = bigA.tile([128, C], F16, tag="xTn", name="xTn",
                                        bufs=2)
                        nc.vector.tensor_copy(xTn[:], ps2[:])
                        nc.tensor.matmul(ec[0][:], xTn[:, 0:128], xTn[:],
                                         start=(nt == 0), stop=(nt == 31))
                        nc.tensor.matmul(ec[1][:, 128:256], xTn[:, 128:256],
                                         xTn[:, 128:256],
                                         start=(nt == 0), stop=(nt == 31))

                    # fill ec[1]'s off-diagonal block by symmetry
                    ecsb = work.tile([128, 128], F32, tag="ecsb", name="ecsb")
                    nc.vector.tensor_copy(ecsb[:], ec[0][:, 128:256])
                    identf = work.tile([128, 128], F32, tag="identf", name="identf")
                    nc.vector.tensor_copy(identf[:], ident)
                    nc.tensor.transpose(ec[1][:, 0:128], ecsb[:], identf[:])

                    # channel-attention softmax (row-wise) + transpose
                    U = [None, None]
                    for ih in range(2):
                        negmax = work.tile([128, 1], F32, tag="negmax",
                                           name="negmax")
                        nc.vector.reduce_max(negmax[:], ec[ih][:], axis=AX.X,
                                             negate=True)
                        U[ih] = bigA.tile([128, C], F16, tag=f"U{ih}",
                                          name=f"U{ih}")
                        nc.scalar.activation(U[ih][:], ec[ih][:], AF.Exp,
                                             bias=negmax[:], scale=1.0)
                        ssum = work.tile([128, 1], F32, tag="ssum", name="ssum")
                        nc.vector.reduce_sum(ssum[:], U[ih][:], axis=AX.X)
                        rc = work.tile([128, 1], F32, tag="rc", name="rc")
                        nc.vector.reciprocal(rc[:], ssum[:])
                        grca[ih] = consts.tile([128, 1], F32, tag=f"grca{ih}",
                                               name=f"grca{ih}")
                        nc.vector.tensor_scalar_mul(grca[ih][:], rc[:], gca)
                    for ih in range(2):
                        for jt in range(2):
                            trp = psAB.tile([128, 128], F16, tag="xt", name="utr",
                                            bufs=2)
                            nc.tensor.transpose(
                                trp[:], U[ih][:, 128 * jt : 128 * jt + 128],
                                ident)
                            nc.vector.tensor_copy(A[:, jt, 128 * ih : 128 * ih + 128],
                                                  trp[:])

                # ---------- phase C: channel-attn apply + position attention ----
                pad = [persist.tile([128, WROWS, W + 2], BF16, tag=f"pad{t}",
                                    name=f"pad{t}")
                       for t in range(4)]
                # only the two border columns need zeroing: the finalize ops
                # write every interior column of all 34 rows
                zcol = work.tile([128, WROWS, 1], F32, tag="zcol", name="zcol")
                nc.vector.memset(zcol[:], 0.0)
                for t in range(4):
                    nc.vector.tensor_copy(pad[t][:, :, 0:1], zcol[:])
                    nc.vector.tensor_copy(pad[t][:, :, W + 1 : W + 2], zcol[:])

                with (
                    tc.tile_pool(name="psC", bufs=1, space="PSUM") as psC,
                    tc.tile_pool(name="ptp", bufs=2) as ptp,
                ):
                    # ca = (U @ xq) * (gamma_ca / rowsum) + xq, into padded tiles
                    for ih in range(2 * (not os.environ.get("KERNEL_SKIP_CA"))):
                        isl = slice(128 * ih, 128 * ih + 128)
                        for off, cw in CHUNKS:
                            rows = cw // W
                            roff = off // W
                            ca = psC.tile([128, cw], F32, tag=f"pa{ih}",
                                          name=f"pa{ih}")
                            nc.tensor.matmul(ca[:], A[:, 0, isl],
                                             xqr[0][:, off : off + cw],
                                             start=True, stop=False)
                            nc.tensor.matmul(ca[:], A[:, 1, isl],
                                             xqr[1][:, off : off + cw],
                                             start=False, stop=True)
                            nc.vector.scalar_tensor_tensor(
                                out=pad[2 + ih][:, roff : roff + rows, 1 : 1 + W],
                                in0=ca[:].rearrange("p (r w) -> p r w", w=W),
                                scalar=grca[ih][:],
                                in1=xqr[ih][:, off : off + cw]
                                    .rearrange("p (r w) -> p r w", w=W),
                                op0=ALU.mult, op1=ALU.add,
                            )

                    for off, cw in ([] if os.environ.get("KERNEL_SKIP_C") else CHUNKS):
                        PT = ptp.tile([128, 32, 512], BF16, tag="pt", name="pt")
                        for g in range(16):
                            eg = psC.tile([128, 2, 512], F32, tag=f"eg{g % 2}",
                                          name=f"eg{g % 2}")
                            for jj in range(2):
                                jt = 2 * g + jj
                                nc.tensor.matmul(
                                    eg[:, jj, 0:cw],
                                    krep[:, 128 * jt : 128 * jt + 128],
                                    qrep[:, off : off + cw],
                                    start=True, stop=True,
                                )
                            nc.scalar.activation(PT[:, 2 * g : 2 * g + 2, 0:cw],
                                                 eg[:, :, 0:cw], AF.Exp,
                                                 bias=0.0, scale=1.0)

                        for ib in range(cw // 128):
                            gib = off // 128 + ib
                            paps = psC.tile([128, 257], F32, tag=f"pa{ib % 3}",
                                            name=f"pa{ib % 3}")
                            for jt in range(32):
                                nc.tensor.matmul(
                                    paps[:],
                                    PT[:, jt, 128 * ib : 128 * ib + 128],
                                    vT[:, jt, :],
                                    start=(jt == 0), stop=(jt == 31),
                                )
                            recip = work.tile([128, 1], F32, tag="recip",
                                              name="recip", bufs=2)
                            nc.vector.reciprocal(recip[:], paps[:, 256:257])
                            grm = work.tile([128, 1], F32, tag="grm", name="grm",
                                            bufs=2)
                            nc.vector.tensor_scalar_mul(grm[:], recip[:], gpa)
                            tsc = work.tile([128, C], F16, tag="tsc", name="tsc",
                                            bufs=2)
                            nc.vector.tensor_scalar_mul(tsc[:], paps[:, 0:C],
                                                        grm[:])
                            for ch in range(2):
                                trp = psC.tile([128, 128], F16, tag="tr",
                                               name="tr")
                                nc.tensor.transpose(
                                    trp[:], tsc[:, 128 * ch : 128 * ch + 128],
                                    ident)
                                r2 = 128 // W
                                r0 = gib * r2
                                gbv = smalls[:, S_GBV0 + ch : S_GBV0 + ch + 1]
                                nc.vector.scalar_tensor_tensor(
                                    out=pad[ch][:, r0 : r0 + r2, 1 : 1 + W],
                                    in0=trp[:].rearrange("p (r w) -> p r w", w=W),
                                    scalar=gbv,
                                    in1=xqr[ch][:, 128 * gib : 128 * gib + 128]
                                        .rearrange("p (r w) -> p r w", w=W),
                                    op0=ALU.add, op1=ALU.add,
                                )

                    # zero out-of-image halo rows of the position-attention pads
                    # (they carry bias terms from the zero-padded xq window)
                    for ch in range(2):
                        nc.vector.tensor_scalar_mul(
                            pad[ch][:, 0:1, :], pad[ch][:, 0:1, :],
                            smalls[:, S_MTOP:S_MTOP + 1])
                        nc.vector.tensor_scalar_mul(
                            pad[ch][:, WROWS - 1 : WROWS, :],
                            pad[ch][:, WROWS - 1 : WROWS, :],
                            smalls[:, S_MBOT:S_MBOT + 1])

            # ---------- phase D: 3x3 conv + BN stats ----------
            y_sb = [persist.tile([128, 2048], F32, tag=f"ysb{o}", name=f"ysb{o}")
                    for o in range(2)]
            allst = [None, None]
            if os.environ.get("KERNEL_SKIP_D"):
                for o in range(2):
                    nc.vector.memset(y_sb[o][:], 0.0)
            sums = [consts.tile([128, 4], F32, tag=f"sums{o}", name=f"sums{o}")
                    for o in range(2)]
            sqs = [consts.tile([128, 4], F32, tag=f"sqs{o}", name=f"sqs{o}")
                   for o in range(2)]
            if os.environ.get("KERNEL_SKIP_D"):
                for o in range(2):
                    nc.vector.memset(sums[o][:], 0.0)
                    nc.vector.memset(sqs[o][:], 0.0)

            with (
                tc.tile_pool(name="psD", bufs=4, space="PSUM") as psD,
            ):
                for oh in range(2 * (not os.environ.get("KERNEL_SKIP_D"))):
                    for pc in range(4):
                        yps = psD.tile([128, 512], F32, tag="y", name="y")
                        first = True
                        for it in range(4):
                            for dy in range(3):
                                for dx in range(3):
                                    woff = (it * 18 + (dy * 3 + dx) * 2 + oh) * 128
                                    last = (it == 3 and dy == 2 and dx == 2)
                                    rhs = pad[it][:, 8 * pc + dy : 8 * pc + dy + 8,
                                                  dx : dx + W]
                                    nc.tensor.matmul(
                                        yps[:], wfp_sb[:, woff : woff + 128], rhs,
                                        start=first, stop=last,
                                    )
                                    first = False
                        ysl = y_sb[oh][:, 512 * pc : 512 * pc + 512]
                        nc.scalar.copy(ysl, yps[:])
                        nc.vector.reduce_sum(sums[oh][:, pc : pc + 1], yps[:],
                                             axis=AX.X)
                        dscr = work.tile([128, 512], F32, tag="dscr", name="dscr",
                                         bufs=2)
                        nc.scalar.activation(dscr[:], yps[:], AF.Square,
                                             accum_out=sqs[oh][:, pc : pc + 1])

                    stats_sb = consts.tile([128, 2], F32, tag=f"stats{oh}",
                                           name=f"stats{oh}")
                    nc.vector.reduce_sum(stats_sb[:, 0:1], sums[oh][:], axis=AX.X)
                    nc.vector.reduce_sum(stats_sb[:, 1:2], sqs[oh][:], axis=AX.X)
                    nc.sync.dma_start(out=stats_in_d[oh][:], in_=stats_sb[:])
                    # oh=0's AllReduce overlaps oh=1's conv half
                    nc.gpsimd.collective_compute(
                        "AllReduce", ALU.add,
                        replica_groups=[list(range(8))],
                        ins=[stats_in_d[oh][:]],
                        outs=[stats_out_d[oh][:]],
                    )
                    allst[oh] = consts.tile([128, 2], F32, tag=f"allst{oh}",
                                            name=f"allst{oh}")
                    nc.sync.dma_start(out=allst[oh][:], in_=stats_out_d[oh][:])

            scale_t = [None, None]
            shift_t = [None, None]
            for oh in range(2):
                mean = work.tile([128, 1], F32, tag="mean", name="mean")
                nc.vector.tensor_scalar_mul(mean[:], allst[oh][:, 0:1], 1.0 / NPOS)
                ex2 = work.tile([128, 1], F32, tag="ex2", name="ex2")
                nc.vector.tensor_scalar_mul(ex2[:], allst[oh][:, 1:2], 1.0 / NPOS)
                msq = work.tile([128, 1], F32, tag="msq", name="msq")
                nc.vector.tensor_mul(msq[:], mean[:], mean[:])
                var = work.tile([128, 1], F32, tag="var", name="var")
                nc.vector.tensor_sub(var[:], ex2[:], msq[:])
                std = work.tile([128, 1], F32, tag="std", name="std")
                nc.scalar.activation(std[:], var[:], AF.Sqrt,
                                     bias=smalls[:, S_EPS:S_EPS + 1], scale=1.0)
                rstd = work.tile([128, 1], F32, tag="rstd", name="rstd")
                nc.vector.reciprocal(rstd[:], std[:])
                scale_t[oh] = consts.tile([128, 1], F32, tag=f"scale{oh}",
                                          name=f"scale{oh}")
                nc.vector.tensor_mul(scale_t[oh][:],
                                     smalls[:, S_BNG0 + oh : S_BNG0 + oh + 1],
                                     rstd[:])
                tmp = work.tile([128, 1], F32, tag="tmp", name="tmp")
                nc.vector.tensor_mul(tmp[:], mean[:], scale_t[oh][:])
                shift_t[oh] = consts.tile([128, 1], F32, tag=f"shift{oh}",
                                          name=f"shift{oh}")
                nc.vector.tensor_sub(shift_t[oh][:],
                                     smalls[:, S_BNB0 + oh : S_BNB0 + oh + 1],
                                     tmp[:])

            for oh in range(2):
                for pc in range(4):
                    rsb = work.tile([128, 512], F32, tag="rsb", name="rsb", bufs=2)
                    nc.vector.tensor_scalar(
                        out=rsb[:], in0=y_sb[oh][:, 512 * pc : 512 * pc + 512],
                        scalar1=scale_t[oh][:], scalar2=shift_t[oh][:],
                        op0=ALU.mult, op1=ALU.add)
                    osb = work.tile([128, 512], F32, tag="osb", name="osb", bufs=2)
                    nc.vector.tensor_scalar_max(osb[:], rsb[:], 0.0)
                    nc.sync.dma_start(
                        out=out_d[128 * oh : 128 * oh + 128,
                                  512 * pc : 512 * pc + 512],
                        in_=osb[:],
                    )

    nc.compile()
    return nc


def _ensure_trace_hook():
    try:
        import antenv.axon_hooks  # noqa: F401
        return
    except ImportError:
        pass
    try:
        from trn_agent_boot.trn_boot import _ntff_profile_via_ctypes
    except ImportError:
        return
    mod = types.ModuleType("antenv.axon_hooks")
    try:
        hook = _ntff_profile_via_ctypes("/opt/axon/libaxon_pjrt.so")
    except Exception:
        return
    mod.get_axon_ntff_profile_hook = lambda: hook
    mod.set_axon_ntff_profile_hook = lambda h: None
    sys.modules["antenv.axon_hooks"] = mod


def kernel(x, wq, bq, wk, bk, wv, bv, gamma_pa, gamma_ca, wf, bn_gamma, bn_beta):
    x = np.ascontiguousarray(np.asarray(x, np.float32))
    wq = np.asarray(wq, np.float32)
    bq = np.asarray(bq, np.float32)
    wk = np.asarray(wk, np.float32)
    bk = np.asarray(bk, np.float32)
    wv = np.asarray(wv, np.float32)
    bv = np.asarray(bv, np.float32)
    wf = np.asarray(wf, np.float32)
    gpa = float(np.asarray(gamma_pa).reshape(-1)[0])
    gca = float(np.asarray(gamma_ca).reshape(-1)[0])
    bn_gamma = np.asarray(bn_gamma, np.float32)
    bn_beta = np.asarray(bn_beta, np.float32)

    nc = _build(gpa, gca)

    # shared (per-core-identical) packed weights, all fp16
    wqrep = np.tile(wq.T, (1, 4))                # [256, 128]
    wkrep = np.tile(wk.T, (1, 4))
    wvt = wv.T                                   # [256, 256]
    wpack = np.zeros((128, WPACK_COLS), np.float32)
    for ct in range(2):
        rows = slice(128 * ct, 128 * ct + 128)
        wpack[:, ct * CT1 + W_Q : ct * CT1 + W_Q + 128] = wqrep[rows]
        wpack[:, ct * CT1 + W_K : ct * CT1 + W_K + 128] = wkrep[rows]
        wpack[:, ct * CT1 + W_V : ct * CT1 + W_V + C] = wvt[rows]
    wpack[:, IDN:IDN + 128] = np.eye(128, dtype=np.float32)
    wpack_h = np.ascontiguousarray(wpack).astype(np.float16)

    # wfp[i, it*2304 + (dy*3+dx)*2*128 + oh*128 + o'] = wf[o, 128*it + i, dy, dx]
    wft = np.ascontiguousarray(
        wf.reshape(C, 4, 128, 3, 3).transpose(1, 2, 3, 4, 0).reshape(4, 128, 2304))
    wfp_h = np.ascontiguousarray(
        wft.transpose(1, 0, 2).reshape(128, 4 * 2304)).astype(ml_dtypes.bfloat16)

    xh = x.astype(np.float16)

    in_maps = []
    for core in range(8):
        b, hf = divmod(core, 2)
        r0 = hf * 32
        e0 = r0 - 1
        xq = np.zeros((C, WROWS, W), np.float16)
        lo, hi = max(e0, 0), min(e0 + WROWS, H)
        xq[:, lo - e0 : hi - e0, :] = xh[b][:, lo:hi, :]
        smalls = np.zeros((128, SMALL_COLS), np.float32)
        smalls[:, S_BQ] = np.tile(bq, 4) / 4.0
        smalls[:, S_BK] = np.tile(bk, 4)
        smalls[:, S_GBV0] = gpa * bv[0:128]
        smalls[:, S_GBV1] = gpa * bv[128:256]
        smalls[:, S_BNG0] = bn_gamma[0:128]
        smalls[:, S_BNG1] = bn_gamma[128:256]
        smalls[:, S_BNB0] = bn_beta[0:128]
        smalls[:, S_BNB1] = bn_beta[128:256]
        smalls[:, S_MTOP] = 0.0 if hf == 0 else 1.0
        smalls[:, S_MBOT] = 1.0 if hf == 0 else 0.0
        smalls[:, S_EPS] = BN_EPS
        in_maps.append({
            "xs": np.ascontiguousarray(xh[b].reshape(C, N)),
            "xq": np.ascontiguousarray(xq.reshape(C, WQ)),
            "wpack": wpack_h,
            "smalls": np.ascontiguousarray(smalls),
            "wfp": wfp_h,
        })

    trace = bool(os.environ.get("BASS_TRACE"))
    if trace:
        _ensure_trace_hook()
    res = run_bass_kernel_spmd(nc, in_maps, list(range(8)), trace=trace)
    LAST_RESULT["exec_time_ns"] = res.exec_time_ns
    LAST_RESULT["mean_exec_time_ns"] = res.mean_exec_time_ns

    out = np.empty((B, C, H, W), np.float32)
    for core in range(8):
        b, hf = divmod(core, 2)
        out[b][:, 32 * hf : 32 * hf + 32, :] = (
            res.results[core]["out"].reshape(C, 32, W)
        )
    return out


# revision 29
# speedup vs baseline: 1.0548x; 1.0422x over previous
"""Trainium2 Bass kernel for a DANet-style DualAttentionBlock.

Full-input contract: kernel(**inputs) takes the complete tensors and returns
the complete [4, 256, 64, 64] output.

Sharding: 8 NeuronCores = 4 samples x 2 row-halves (32 rows each). Each core:
  - computes q for its 34-row query window (32 real + 1 halo row each side;
    out-of-range halo rows are zero-padded on host and zeroed on device after
    the attention combine),
  - computes k/v for all 4096 key positions of its sample,
  - position attention in transposed orientation (keys on partitions) so the
    softmax denominator folds into matmuls and no attention transpose is
    needed,
  - full channel attention (softmax row-wise, then a 256x256 PE transpose;
    the symmetric energy matrix lets the lower-left block be a transpose of
    the upper-right),
  - the 3x3 fusion conv for its 32 output rows (halo rows give exact edges),
  - partial BN stats; two pipelined AllReduces across the 8 cores yield exact
    train-mode batch stats; normalize + ReLU on device.

Precision strategy: fp16 operands everywhere on the attention path (q/k/v/x
and channel energies; fp32 PSUM accumulation throughout), which keeps every
PE weight load at the fast 2-byte path; attention weights and the fusion conv
in bf16. The PV matmul runs in [query, channel] orientation against V
augmented with a ones column, so the softmax denominator falls out of column
256 for free; results are PE-transposed back into the conv layout. All small
per-partition vectors (biases, BN affine, halo masks) ride in one packed
[128, 12] tensor, and the matmul weights in one packed fp16 tensor, so the
startup-critical path is six large DMAs split across both hardware queues.
"""

import os
import sys
import types

for _p in ("/opt/trn_rl_repo",):
    if _p not in sys.path:
        sys.path.append(_p)

import numpy as np
import ml_dtypes  # noqa: F401

import concourse.bass as bass
import concourse.mybir as mybir
import concourse.tile as tile
from concourse import bacc
from concourse.bass_utils import run_bass_kernel_spmd

F32 = mybir.dt.float32
BF16 = mybir.dt.bfloat16
F16 = mybir.dt.float16
AF = mybir.ActivationFunctionType
ALU = mybir.AluOpType
AX = mybir.AxisListType

B, C, H, W = 4, 256, 64, 64
N = H * W              # 4096 key positions
WROWS = 34             # query-window rows (1 halo + 32 real + 1 halo)
WQ = WROWS * W         # 2176 window positions
BN_EPS = 1e-5
NPOS = float(B * H * W)  # BN normalizer (16384)

# i-chunks for the attention phase
CHUNKS = [(0, 512), (512, 512), (1024, 512), (1536, 384), (1920, 256)]

# packed weight tensor column offsets (all fp16); ct=1 block lives at +1152
W_Q, W_K, W_V, IDN = 0, 128, 256, 512
CT1 = 1152
WPACK_COLS = 2304

# packed smalls column offsets (f32)
S_BQ, S_BK = 0, 1
S_GBV0, S_GBV1 = 2, 3
S_BNG0, S_BNG1 = 4, 5
S_BNB0, S_BNB1 = 6, 7
S_MTOP, S_MBOT = 8, 9
S_EPS = 10
SMALL_COLS = 11

LAST_RESULT = {}


def _build(gpa: float, gca: float):
    nc = bacc.Bacc("TRN2", target_bir_lowering=False)

    xs_d = nc.dram_tensor("xs", [C, N], F16, kind="ExternalInput")
    xq_d = nc.dram_tensor("xq", [C, WQ], F16, kind="ExternalInput")
    wpack_d = nc.dram_tensor("wpack", [128, WPACK_COLS], F16, kind="ExternalInput")
    smalls_d = nc.dram_tensor("smalls", [128, SMALL_COLS], F32, kind="ExternalInput")
    wfp_d = nc.dram_tensor("wfp", [128, 4 * 2304], BF16, kind="ExternalInput")
    out_d = nc.dram_tensor("out", [C, 2048], F32, kind="ExternalOutput")

    stats_in_d = [nc.dram_tensor(f"stats_in{o}", [128, 2], F32) for o in range(2)]
    stats_out_d = [nc.dram_tensor(f"stats_out{o}", [128, 2], F32,
                                  addr_space="Shared") for o in range(2)]
    dummy_in_d = nc.dram_tensor("dummy_in", [1, 16], F32)
    dummy_out_d = nc.dram_tensor("dummy_out", [1, 16], F32, addr_space="Shared")

    with tile.TileContext(nc) as tc:
        with (
            tc.tile_pool(name="consts", bufs=1) as consts,
            tc.tile_pool(name="work", bufs=1) as work,
            tc.tile_pool(name="persist", bufs=1) as persist,
        ):
            # long-lived activation tensors for the attention phase
            with tc.tile_pool(name="bigC", bufs=1) as bigC:
                qrep = bigC.tile([128, WQ], F16, tag="qrep", name="qrep")
                krep = bigC.tile([128, N], F16, tag="krep", name="krep")
                vT = bigC.tile([128, 32, 257], BF16, tag="vT", name="vT")
                A = persist.tile([128, 2, C], F16, tag="A", name="A")
                grca = [None, None]
                xqr = [None, None]

                # ---------- phase A/B: input DMA, qkv gen, xT, channel attn ----
                with (
                    tc.tile_pool(name="bigA", bufs=1) as bigA,
                    tc.tile_pool(name="psAB", bufs=1, space="PSUM") as psAB,
                ):
                    # startup-critical loads, split over both HWDGE queues;
                    # x halves are split along n so k/v-gen can start on the
                    # first half while the second still streams
                    smalls = consts.tile([128, SMALL_COLS], F32, tag="smalls",
                                         name="smalls")
                    nc.sync.dma_start(out=smalls[:], in_=smalls_d[:])
                    wpk = consts.tile([128, WPACK_COLS], F16, tag="wpk", name="wpk")
                    nc.sync.dma_start(out=wpk[:], in_=wpack_d[:])
                    for ct in range(2):
                        xqr[ct] = bigC.tile([128, WQ], F16, tag=f"xqr{ct}",
                                            name=f"xqr{ct}")
                    nc.scalar.dma_start(out=xqr[1][:], in_=xq_d[128:256, :])
                    nc.sync.dma_start(out=xqr[0][:], in_=xq_d[0:128, :])
                    xr = [None, None]
                    for ct in range(2):
                        xr[ct] = bigA.tile([128, N], F16, tag=f"xr{ct}",
                                           name=f"xr{ct}")
                    nc.sync.dma_start(out=xr[0][:, 0:2048], in_=xs_d[0:128, 0:2048])
                    nc.scalar.dma_start(out=xr[1][:, 0:2048],
                                        in_=xs_d[128:256, 0:2048])
                    nc.sync.dma_start(out=xr[0][:, 2048:4096],
                                      in_=xs_d[0:128, 2048:4096])
                    nc.scalar.dma_start(out=xr[1][:, 2048:4096],
                                        in_=xs_d[128:256, 2048:4096])

                    wq_r = [wpk[:, ct * CT1 + W_Q : ct * CT1 + W_Q + 128]
                            for ct in range(2)]
                    wk_r = [wpk[:, ct * CT1 + W_K : ct * CT1 + W_K + 128]
                            for ct in range(2)]
                    wv_r = [wpk[:, ct * CT1 + W_V : ct * CT1 + W_V + C]
                            for ct in range(2)]
                    ident = wpk[:, IDN:IDN + 128]

                    # conv weights prefetch (needed only in phase D)
                    wfp_sb = persist.tile([128, 4 * 2304], BF16, tag="wfp",
                                          name="wfp")
                    nc.scalar.dma_start(out=wfp_sb[:], in_=wfp_d[:])

                    # early throwaway AllReduce: absorbs inter-core skew and
                    # first-use collective setup cost so the BN-stats
                    # AllReduces at the end run at their minimum latency
                    dummy_sb = work.tile([1, 16], F32, tag="dummy", name="dummy")
                    nc.vector.memset(dummy_sb[:], 0.0)
                    nc.gpsimd.dma_start(out=dummy_in_d[:], in_=dummy_sb[:])
                    nc.gpsimd.collective_compute(
                        "AllReduce", ALU.add,
                        replica_groups=[list(range(8))],
                        ins=[dummy_in_d[:]],
                        outs=[dummy_out_d[:]],
                    )

                    # ones column of augmented V: softmax denominator source
                    nc.vector.memset(vT[:, :, 256:257], 1.0)

                    # q/k generation (4x-replicated along d)
                    for off, cw in CHUNKS:
                        ps = psAB.tile([128, cw], F32, tag="qk", name="qk", bufs=2)
                        nc.tensor.matmul(ps[:], wq_r[0], xqr[0][:, off : off + cw],
                                         start=True, stop=False)
                        nc.tensor.matmul(ps[:], wq_r[1], xqr[1][:, off : off + cw],
                                         start=False, stop=True)
                        nc.scalar.activation(qrep[:, off : off + cw], ps[:],
                                             AF.Identity,
                                             bias=smalls[:, S_BQ:S_BQ + 1],
                                             scale=0.25)
                    for kc in range(8):
                        off = 512 * kc
                        ps = psAB.tile([128, 512], F32, tag="qk", name="qk", bufs=2)
                        nc.tensor.matmul(ps[:], wk_r[0], xr[0][:, off : off + 512],
                                         start=True, stop=False)
                        nc.tensor.matmul(ps[:], wk_r[1], xr[1][:, off : off + 512],
                                         start=False, stop=True)
                        nc.scalar.activation(krep[:, off : off + 512], ps[:],
                                             AF.Identity,
                                             bias=smalls[:, S_BK:S_BK + 1],
                                             scale=1.0)

                    # vT gen + x transpose -> channel-attention energy
                    # ec[1] accumulates only its diagonal block; the off-diag
                    # block arrives by transposing ec[0]'s right half.
                    ec = [psAB.tile([128, C], F32, tag=f"ec{ih}", name=f"ec{ih}")
                          for ih in range(2)]
                    for nt in range(32):
                        sl = slice(128 * nt, 128 * nt + 128)
                        ps = psAB.tile([128, C], F32, tag="vx", name="vx", bufs=2)
                        nc.tensor.matmul(ps[:], xr[0][:, sl], wv_r[0],
                                         start=True, stop=False)
                        nc.tensor.matmul(ps[:], xr[1][:, sl], wv_r[1],
                                         start=False, stop=True)
                        nc.vector.tensor_copy(vT[:, nt, 0:256], ps[:])
                        ps2 = psAB.tile([128, C], F16, tag="xt", name="xt", bufs=2)
                        nc.tensor.transpose(ps2[:, 0:128], xr[0][:, sl], ident)
                        nc.tensor.transpose(ps2[:, 128:256], xr[1][:, sl], ident)
                        xTn = bigA.tile([128, C], F16, tag="xTn", name="xTn",
                                        bufs=2)
                        nc.vector.tensor_copy(xTn[:], ps2[:])
                        nc.tensor.matmul(ec[0][:], xTn[:, 0:128], xTn[:],
                                         start=(nt == 0), stop=(nt == 31))
                        nc.tensor.matmul(ec[1][:, 128:256], xTn[:, 128:256],
                                         xTn[:, 128:256],
                                         start=(nt == 0), stop=(nt == 31))

                    # fill ec[1]'s off-diagonal block by symmetry
                    ecsb = work.tile([128, 128], F32, tag="ecsb", name="ecsb")
                    nc.vector.tensor_copy(ecsb[:], ec[0][:, 128:256])
                    identf = work.tile([128, 128], F32, tag="identf", name="identf")
                    nc.vector.tensor_copy(identf[:], ident)
                    nc.tensor.transpose(ec[1][:, 0:128], ecsb[:], identf[:])

                    # channel-attention softmax (row-wise) + transpose
                    U = [None, None]
                    for ih in range(2):
                        negmax = work.tile([128, 1], F32, tag="negmax",
                                           name="negmax")
                        nc.vector.reduce_max(negmax[:], ec[ih][:], axis=AX.X,
                                             negate=True)
                        U[ih] = bigA.tile([128, C], F16, tag=f"U{ih}",
                                          name=f"U{ih}")
                        nc.scalar.activation(U[ih][:], ec[ih][:], AF.Exp,
                                             bias=negmax[:], scale=1.0)
                        ssum = work.tile([128, 1], F32, tag="ssum", name="ssum")
                        nc.vector.reduce_sum(ssum[:], U[ih][:], axis=AX.X)
                        rc = work.tile([128, 1], F32, tag="rc", name="rc")
                        nc.vector.reciprocal(rc[:], ssum[:])
                        grca[ih] = consts.tile([128, 1], F32, tag=f"grca{ih}",
                                               name=f"grca{ih}")
                        nc.vector.tensor_scalar_mul(grca[ih][:], rc[:], gca)
                    for ih in range(2):
                        for jt in range(2):
                            trp = psAB.tile([128, 128], F16, tag="xt", name="utr",
                                            bufs=2)
                            nc.tensor.transpose(
                                trp[:], U[ih][:, 128 * jt : 128 * jt + 128],
                                ident)
                            nc.vector.tensor_copy(A[:, jt, 128 * ih : 128 * ih + 128],
                                                  trp[:])

                # ---------- phase C: channel-attn apply + position attention ----
                pad = [persist.tile([128, WROWS, W + 2], BF16, tag=f"pad{t}",
                                    name=f"pad{t}")
                       for t in range(4)]
                # only the two border columns need zeroing: the finalize ops
                # write every interior column of all 34 rows
                zcol = work.tile([128, WROWS, 1], F32, tag="zcol", name="zcol")
                nc.vector.memset(zcol[:], 0.0)
                for t in range(4):
                    nc.vector.tensor_copy(pad[t][:, :, 0:1], zcol[:])
                    nc.vector.tensor_copy(pad[t][:, :, W + 1 : W + 2], zcol[:])

                with (
                    tc.tile_pool(name="psC", bufs=1, space="PSUM") as psC,
                    tc.tile_pool(name="ptp", bufs=2) as ptp,
                ):
                    def ca_apply():
                        # ca = (U @ xq) * (gamma_ca / rowsum) + xq, into pads
                        for ih in range(2 * (not os.environ.get("KERNEL_SKIP_CA"))):
                            isl = slice(128 * ih, 128 * ih + 128)
                            for off, cw in CHUNKS:
                                rows = cw // W
                                roff = off // W
                                ca = psC.tile([128, cw], F32, tag=f"pa{ih}",
                                              name=f"pa{ih}")
                                nc.tensor.matmul(ca[:], A[:, 0, isl],
                                                 xqr[0][:, off : off + cw],
                                                 start=True, stop=False)
                                nc.tensor.matmul(ca[:], A[:, 1, isl],
                                                 xqr[1][:, off : off + cw],
                                                 start=False, stop=True)
                                nc.vector.scalar_tensor_tensor(
                                    out=pad[2 + ih][:, roff : roff + rows,
                                                    1 : 1 + W],
                                    in0=ca[:].rearrange("p (r w) -> p r w", w=W),
                                    scalar=grca[ih][:],
                                    in1=xqr[ih][:, off : off + cw]
                                        .rearrange("p (r w) -> p r w", w=W),
                                    op0=ALU.mult, op1=ALU.add,
                                )

                    if os.environ.get("KERNEL_SKIP_C"):
                        ca_apply()
                    for ci, (off, cw) in enumerate(
                            [] if os.environ.get("KERNEL_SKIP_C") else CHUNKS):
                        PT = ptp.tile([128, 32, 512], BF16, tag="pt", name="pt")
                        for g in range(16):
                            eg = psC.tile([128, 2, 512], F32, tag=f"eg{g % 2}",
                                          name=f"eg{g % 2}")
                            for jj in range(2):
                                jt = 2 * g + jj
                                nc.tensor.matmul(
                                    eg[:, jj, 0:cw],
                                    krep[:, 128 * jt : 128 * jt + 128],
                                    qrep[:, off : off + cw],
                                    start=True, stop=True,
                                )
                            nc.scalar.activation(PT[:, 2 * g : 2 * g + 2, 0:cw],
                                                 eg[:, :, 0:cw], AF.Exp,
                                                 bias=0.0, scale=1.0)
                        if ci == 0:
                            # emitted after chunk-0 energies: the PE chews
                            # those while the channel softmax chain finishes
                            ca_apply()

                        for ib in range(cw // 128):
                            gib = off // 128 + ib
                            paps = psC.tile([128, 257], F32, tag=f"pa{ib % 3}",
                                            name=f"pa{ib % 3}")
                            for jt in range(32):
                                nc.tensor.matmul(
                                    paps[:],
                                    PT[:, jt, 128 * ib : 128 * ib + 128],
                                    vT[:, jt, :],
                                    start=(jt == 0), stop=(jt == 31),
                                )
                            recip = work.tile([128, 1], F32, tag="recip",
                                              name="recip", bufs=2)
                            nc.vector.reciprocal(recip[:], paps[:, 256:257])
                            grm = work.tile([128, 1], F32, tag="grm", name="grm",
                                            bufs=2)
                            nc.vector.tensor_scalar_mul(grm[:], recip[:], gpa)
                            tsc = work.tile([128, C], F16, tag="tsc", name="tsc",
                                            bufs=2)
                            nc.vector.tensor_scalar_mul(tsc[:], paps[:, 0:C],
                                                        grm[:])
                            for ch in range(2):
                                trp = psC.tile([128, 128], F16, tag="tr",
                                               name="tr")
                                nc.tensor.transpose(
                                    trp[:], tsc[:, 128 * ch : 128 * ch + 128],
                                    ident)
                                r2 = 128 // W
                                r0 = gib * r2
                                gbv = smalls[:, S_GBV0 + ch : S_GBV0 + ch + 1]
                                nc.vector.scalar_tensor_tensor(
                                    out=pad[ch][:, r0 : r0 + r2, 1 : 1 + W],
                                    in0=trp[:].rearrange("p (r w) -> p r w", w=W),
                                    scalar=gbv,
                                    in1=xqr[ch][:, 128 * gib : 128 * gib + 128]
                                        .rearrange("p (r w) -> p r w", w=W),
                                    op0=ALU.add, op1=ALU.add,
                                )

                    # zero out-of-image halo rows of the position-attention pads
                    # (they carry bias terms from the zero-padded xq window)
                    for ch in range(2):
                        nc.vector.tensor_scalar_mul(
                            pad[ch][:, 0:1, :], pad[ch][:, 0:1, :],
                            smalls[:, S_MTOP:S_MTOP + 1])
                        nc.vector.tensor_scalar_mul(
                            pad[ch][:, WROWS - 1 : WROWS, :],
                            pad[ch][:, WROWS - 1 : WROWS, :],
                            smalls[:, S_MBOT:S_MBOT + 1])

            # ---------- phase D: 3x3 conv + BN stats ----------
            y_sb = [persist.tile([128, 2048], F32, tag=f"ysb{o}", name=f"ysb{o}")
                    for o in range(2)]
            allst = [None, None]
            if os.environ.get("KERNEL_SKIP_D"):
                for o in range(2):
                    nc.vector.memset(y_sb[o][:], 0.0)
            sums = [consts.tile([128, 4], F32, tag=f"sums{o}", name=f"sums{o}")
                    for o in range(2)]
            sqs = [consts.tile([128, 4], F32, tag=f"sqs{o}", name=f"sqs{o}")
                   for o in range(2)]
            if os.environ.get("KERNEL_SKIP_D"):
                for o in range(2):
                    nc.vector.memset(sums[o][:], 0.0)
                    nc.vector.memset(sqs[o][:], 0.0)

            with (
                tc.tile_pool(name="psD", bufs=4, space="PSUM") as psD,
            ):
                for oh in range(2 * (not os.environ.get("KERNEL_SKIP_D"))):
                    for pc in range(4):
                        yps = psD.tile([128, 512], F32, tag="y", name="y")
                        first = True
                        for it in range(4):
                            for dy in range(3):
                                for dx in range(3):
                                    woff = (it * 18 + (dy * 3 + dx) * 2 + oh) * 128
                                    last = (it == 3 and dy == 2 and dx == 2)
                                    rhs = pad[it][:, 8 * pc + dy : 8 * pc + dy + 8,
                                                  dx : dx + W]
                                    nc.tensor.matmul(
                                        yps[:], wfp_sb[:, woff : woff + 128], rhs,
                                        start=first, stop=last,
                                    )
                                    first = False
                        ysl = y_sb[oh][:, 512 * pc : 512 * pc + 512]
                        nc.scalar.copy(ysl, yps[:])
                        nc.vector.reduce_sum(sums[oh][:, pc : pc + 1], yps[:],
                                             axis=AX.X)
                        dscr = work.tile([128, 512], F32, tag="dscr", name="dscr",
                                         bufs=2)
                        nc.scalar.activation(dscr[:], yps[:], AF.Square,
                                             accum_out=sqs[oh][:, pc : pc + 1])

                    stats_sb = consts.tile([128, 2], F32, tag=f"stats{oh}",
                                           name=f"stats{oh}")
                    nc.vector.reduce_sum(stats_sb[:, 0:1], sums[oh][:], axis=AX.X)
                    nc.vector.reduce_sum(stats_sb[:, 1:2], sqs[oh][:], axis=AX.X)
                    nc.sync.dma_start(out=stats_in_d[oh][:], in_=stats_sb[:])
                    # oh=0's AllReduce overlaps oh=1's conv half
                    nc.gpsimd.collective_compute(
                        "AllReduce", ALU.add,
                        replica_groups=[list(range(8))],
                        ins=[stats_in_d[oh][:]],
                        outs=[stats_out_d[oh][:]],
                    )
                    allst[oh] = consts.tile([128, 2], F32, tag=f"allst{oh}",
                                            name=f"allst{oh}")
                    nc.sync.dma_start(out=allst[oh][:], in_=stats_out_d[oh][:])

            scale_t = [None, None]
            shift_t = [None, None]
            for oh in range(2):
                mean = work.tile([128, 1], F32, tag="mean", name="mean")
                nc.vector.tensor_scalar_mul(mean[:], allst[oh][:, 0:1], 1.0 / NPOS)
                ex2 = work.tile([128, 1], F32, tag="ex2", name="ex2")
                nc.vector.tensor_scalar_mul(ex2[:], allst[oh][:, 1:2], 1.0 / NPOS)
                msq = work.tile([128, 1], F32, tag="msq", name="msq")
                nc.vector.tensor_mul(msq[:], mean[:], mean[:])
                var = work.tile([128, 1], F32, tag="var", name="var")
                nc.vector.tensor_sub(var[:], ex2[:], msq[:])
                std = work.tile([128, 1], F32, tag="std", name="std")
                nc.scalar.activation(std[:], var[:], AF.Sqrt,
                                     bias=smalls[:, S_EPS:S_EPS + 1], scale=1.0)
                rstd = work.tile([128, 1], F32, tag="rstd", name="rstd")
                nc.vector.reciprocal(rstd[:], std[:])
                scale_t[oh] = consts.tile([128, 1], F32, tag=f"scale{oh}",
                                          name=f"scale{oh}")
                nc.vector.tensor_mul(scale_t[oh][:],
                                     smalls[:, S_BNG0 + oh : S_BNG0 + oh + 1],
                                     rstd[:])
                tmp = work.tile([128, 1], F32, tag="tmp", name="tmp")
                nc.vector.tensor_mul(tmp[:], mean[:], scale_t[oh][:])
                shift_t[oh] = consts.tile([128, 1], F32, tag=f"shift{oh}",
                                          name=f"shift{oh}")
                nc.vector.tensor_sub(shift_t[oh][:],
                                     smalls[:, S_BNB0 + oh : S_BNB0 + oh + 1],
                                     tmp[:])

            for oh in range(2):
                for pc in range(4):
                    rsb = work.tile([128, 512], F32, tag="rsb", name="rsb", bufs=2)
                    nc.vector.tensor_scalar(
                        out=rsb[:], in0=y_sb[oh][:, 512 * pc : 512 * pc + 512],
                        scalar1=scale_t[oh][:], scalar2=shift_t[oh][:],
                        op0=ALU.mult, op1=ALU.add)
                    osb = work.tile([128, 512], F32, tag="osb", name="osb", bufs=2)
                    nc.vector.tensor_scalar_max(osb[:], rsb[:], 0.0)
                    nc.sync.dma_start(
                        out=out_d[128 * oh : 128 * oh + 128,
                                  512 * pc : 512 * pc + 512],
                        in_=osb[:],
                    )

    nc.compile()
    return nc


def _ensure_trace_hook():
    try:
        import antenv.axon_hooks  # noqa: F401
        return
    except ImportError:
        pass
    try:
        from trn_agent_boot.trn_boot import _ntff_profile_via_ctypes
    except ImportError:
        return
    mod = types.ModuleType("antenv.axon_hooks")
    try:
        hook = _ntff_profile_via_ctypes("/opt/axon/libaxon_pjrt.so")
    except Exception:
        return
    mod.get_axon_ntff_profile_hook = lambda: hook
    mod.set_axon_ntff_profile_hook = lambda h: None
    sys.modules["antenv.axon_hooks"] = mod


def kernel(x, wq, bq, wk, bk, wv, bv, gamma_pa, gamma_ca, wf, bn_gamma, bn_beta):
    x = np.ascontiguousarray(np.asarray(x, np.float32))
    wq = np.asarray(wq, np.float32)
    bq = np.asarray(bq, np.float32)
    wk = np.asarray(wk, np.float32)
    bk = np.asarray(bk, np.float32)
    wv = np.asarray(wv, np.float32)
    bv = np.asarray(bv, np.float32)
    wf = np.asarray(wf, np.float32)
    gpa = float(np.asarray(gamma_pa).reshape(-1)[0])
    gca = float(np.asarray(gamma_ca).reshape(-1)[0])
    bn_gamma = np.asarray(bn_gamma, np.float32)
    bn_beta = np.asarray(bn_beta, np.float32)

    nc = _build(gpa, gca)

    # shared (per-core-identical) packed weights, all fp16
    wqrep = np.tile(wq.T, (1, 4))                # [256, 128]
    wkrep = np.tile(wk.T, (1, 4))
    wvt = wv.T                                   # [256, 256]
    wpack = np.zeros((128, WPACK_COLS), np.float32)
    for ct in range(2):
        rows = slice(128 * ct, 128 * ct + 128)
        wpack[:, ct * CT1 + W_Q : ct * CT1 + W_Q + 128] = wqrep[rows]
        wpack[:, ct * CT1 + W_K : ct * CT1 + W_K + 128] = wkrep[rows]
        wpack[:, ct * CT1 + W_V : ct * CT1 + W_V + C] = wvt[rows]
    wpack[:, IDN:IDN + 128] = np.eye(128, dtype=np.float32)
    wpack_h = np.ascontiguousarray(wpack).astype(np.float16)

    # wfp[i, it*2304 + (dy*3+dx)*2*128 + oh*128 + o'] = wf[o, 128*it + i, dy, dx]
    wft = np.ascontiguousarray(
        wf.reshape(C, 4, 128, 3, 3).transpose(1, 2, 3, 4, 0).reshape(4, 128, 2304))
    wfp_h = np.ascontiguousarray(
        wft.transpose(1, 0, 2).reshape(128, 4 * 2304)).astype(ml_dtypes.bfloat16)

    xh = x.astype(np.float16)

    in_maps = []
    for core in range(8):
        b, hf = divmod(core, 2)
        r0 = hf * 32
        e0 = r0 - 1
        xq = np.zeros((C, WROWS, W), np.float16)
        lo, hi = max(e0, 0), min(e0 + WROWS, H)
        xq[:, lo - e0 : hi - e0, :] = xh[b][:, lo:hi, :]
        smalls = np.zeros((128, SMALL_COLS), np.float32)
        smalls[:, S_BQ] = np.tile(bq, 4) / 4.0
        smalls[:, S_BK] = np.tile(bk, 4)
        smalls[:, S_GBV0] = gpa * bv[0:128]
        smalls[:, S_GBV1] = gpa * bv[128:256]
        smalls[:, S_BNG0] = bn_gamma[0:128]
        smalls[:, S_BNG1] = bn_gamma[128:256]
        smalls[:, S_BNB0] = bn_beta[0:128]
        smalls[:, S_BNB1] = bn_beta[128:256]
        smalls[:, S_MTOP] = 0.0 if hf == 0 else 1.0
        smalls[:, S_MBOT] = 1.0 if hf == 0 else 0.0
        smalls[:, S_EPS] = BN_EPS
        in_maps.append({
            "xs": np.ascontiguousarray(xh[b].reshape(C, N)),
            "xq": np.ascontiguousarray(xq.reshape(C, WQ)),
            "wpack": wpack_h,
            "smalls": np.ascontiguousarray(smalls),
            "wfp": wfp_h,
        })

    trace = bool(os.environ.get("BASS_TRACE"))
    if trace:
        _ensure_trace_hook()
    res = run_bass_kernel_spmd(nc, in_maps, list(range(8)), trace=trace)
    LAST_RESULT["exec_time_ns"] = res.exec_time_ns
    LAST_RESULT["mean_exec_time_ns"] = res.mean_exec_time_ns

    out = np.empty((B, C, H, W), np.float32)
    for core in range(8):
        b, hf = divmod(core, 2)
        out[b][:, 32 * hf : 32 * hf + 32, :] = (
            res.results[core]["out"].reshape(C, 32, W)
        )
    return out
